# revision 4
# baseline (speedup 1.0000x reference)
"""EvULoss Trainium2 kernel.

Math (see the EvU loss definition):
    evidence = exp(logits); alpha = evidence + 1
    max_alpha  = exp(rowmax(logits)) + 1
    predictions== target  <=>  logits[n, target[n]] == rowmax(logits[n])   (tie-free)
    sum_alpha  = rowsum(exp(logits)) + C
    unc        = C / sum_alpha
Everything downstream of {rowmax, rowsumexp} is O(N) and runs at gather
time on the host; the O(N*C) streaming pass (DMA + exp + reductions) runs
on 8 NeuronCores, data-parallel over N.

Per core: shard [8192, 1000] f32, viewed as 16 blocks of [128p, 4, 1000].
  - one 2 MB DMA per block (HWDGE)
  - VectorE: reduce_max over free dims -> rowmax column slice [128, 4]
  - ScalarE: exp with accum_out       -> rowsumexp columns    [128, 1] x4
Outputs per core: rowmax[128, 64], sumexp[128, 64] with n = col*128 + p.
"""

from contextlib import ExitStack

import numpy as np

import concourse.bacc as bacc
import concourse.tile as tile
from concourse import mybir
from concourse.bass_utils import run_bass_kernel_spmd

N_CORES = 8
N, C = 65536, 1000
NSHARD = N // N_CORES          # 8192 rows per core
P = 128                        # SBUF partitions
BLK = 4                        # 128-row blocks per DMA: [128, 4, 1000] f32 = 2 MB
NT = NSHARD // (P * BLK)       # 16 outer iterations
TCOLS = NSHARD // P            # 64 stat columns
EPS = 1e-10
BETA = 1.0

_NC_CACHE = None


def _build_bass():
    nc = bacc.Bacc("TRN2", target_bir_lowering=False)
    x = nc.dram_tensor("x", [NSHARD, C], mybir.dt.float32, kind="ExternalInput")
    rowmax = nc.dram_tensor(
        "rowmax", [P, TCOLS], mybir.dt.float32, kind="ExternalOutput"
    )
    sumexp = nc.dram_tensor(
        "sumexp", [P, TCOLS], mybir.dt.float32, kind="ExternalOutput"
    )

    # shard row n = t*(BLK*P) + i*P + p  ->  xv[t][p, i, c]
    xv = x.ap().rearrange("(t i p) c -> t p i c", i=BLK, p=P)

    with tile.TileContext(nc) as tc:
        with ExitStack() as ctx:
            xin = ctx.enter_context(tc.tile_pool(name="xin", bufs=4))
            expp = ctx.enter_context(tc.tile_pool(name="expp", bufs=2))
            stats = ctx.enter_context(tc.tile_pool(name="stats", bufs=1))

            smax = stats.tile([P, TCOLS], mybir.dt.float32)
            ssum = stats.tile([P, TCOLS], mybir.dt.float32)

            for t in range(NT):
                xt = xin.tile([P, BLK, C], mybir.dt.float32)
                nc.sync.dma_start(out=xt, in_=xv[t])
                nc.vector.reduce_max(
                    smax[:, t * BLK : (t + 1) * BLK],
                    xt,
                    axis=mybir.AxisListType.X,
                )
                for i in range(BLK):
                    col = t * BLK + i
                    et = expp.tile([P, C], mybir.dt.float32)
                    nc.scalar.activation(
                        et,
                        xt[:, i, :],
                        mybir.ActivationFunctionType.Exp,
                        accum_out=ssum[:, col : col + 1],
                    )

            nc.sync.dma_start(out=rowmax.ap(), in_=smax)
            nc.sync.dma_start(out=sumexp.ap(), in_=ssum)
    nc.compile()
    return nc


def _run_spmd(output_f32, trace=False, **kwargs):
    """Run the streaming pass on 8 cores. Returns (rowmax[N], sumexp[N], results)."""
    global _NC_CACHE
    if _NC_CACHE is None:
        _NC_CACHE = _build_bass()
    nc = _NC_CACHE
    in_maps = [
        {"x": output_f32[c * NSHARD : (c + 1) * NSHARD]} for c in range(N_CORES)
    ]
    res = run_bass_kernel_spmd(
        nc, in_maps, core_ids=list(range(N_CORES)), trace=trace, **kwargs
    )
    # out[p, col] -> shard row col*128 + p
    rowmax = np.concatenate(
        [r["rowmax"].T.reshape(-1) for r in res.results]
    )
    sumexp = np.concatenate(
        [r["sumexp"].T.reshape(-1) for r in res.results]
    )
    return rowmax, sumexp, res


def kernel(output, target, optimal_uncertainty_threshold, num_classes):
    output = np.ascontiguousarray(np.asarray(output), dtype=np.float32)
    target = np.asarray(target).astype(np.int64)
    th = float(np.asarray(optimal_uncertainty_threshold).reshape(-1)[0])
    c = float(int(num_classes))

    rowmax, sumexp, _ = _run_spmd(output)

    max_alpha = np.exp(rowmax.astype(np.float64)) + 1.0
    sum_alpha = sumexp.astype(np.float64) + c
    unc = c / sum_alpha

    umin = unc.min()
    umax = unc.max()
    unc_th = umin + th * (umax - umin)

    picked = output[np.arange(N), target]
    correct = picked == rowmax
    certain = unc <= unc_th
    t = np.tanh(unc)

    n_ac = np.sum(np.where(correct & certain, max_alpha * (1.0 - t), 0.0))
    n_au = np.sum(np.where(correct & ~certain, max_alpha * t, 0.0))
    n_ic = np.sum(np.where(~correct & certain, (1.0 - max_alpha) * (1.0 - t), 0.0))
    n_iu = np.sum(np.where(~correct & ~certain, (1.0 - max_alpha) * t, 0.0))

    evu = (n_ac + n_iu) / (n_ac + n_au + n_ic + n_iu + EPS)
    loss = -BETA * np.log(evu + EPS)
    return np.array([loss], dtype=np.float32)
